# revision 1
# baseline (speedup 1.0000x reference)
"""GQA attention (B=2,T=2048,D=2048, HQ=32, HKV=8, RoPE, full softmax) on 8 trn2 cores.

Sharding: one KV head (+ its 4 Q heads) per core, x replicated; each core
computes its partial W_o product; host sums the 8 partials.

All on-device layouts are transposed (features-on-partitions, tokens-on-free)
so every matmul streams a >=256-wide moving dim in fp32r (1 cycle/row).
Softmax denominator comes for free from a ones-column appended to V.
"""

import os
import sys

import numpy as np

for _p in ("/opt/trn_rl_repo", "/root/.axon_site/_ro/trn_rl_repo"):
    if os.path.isdir(_p) and _p not in sys.path:
        sys.path.append(_p)

import concourse.bacc as bacc
import concourse.bass as bass
import concourse.mybir as mybir
import concourse.tile as tile
from concourse.bass_utils import run_bass_kernel_spmd
from concourse.masks import make_identity

B, T, D = 2, 2048, 2048
HQ, HKV, HD = 32, 8, 64
NH = HQ // HKV        # 4 q heads per core
QF = NH * HD          # 256 q features per core
KF = HD               # 64 k (or v) features per core
BT = B * T            # 4096
P = 128
NCHUNK = 512          # token chunk (moving dim)
NCH = BT // NCHUNK    # 8
KT = D // P           # 16 contraction tiles over D
TBP = T // P          # 16 key tiles per batch
QCH = T // NCHUNK     # 4 q chunks per batch
MB = QF // P          # 2 q-feature blocks
ROPE_BASE = 10000.0
SCALE = 1.0 / 8.0     # 1/sqrt(HD)

f32 = mybir.dt.float32
f32r = mybir.dt.float32r
AF = mybir.ActivationFunctionType
OP = mybir.AluOpType

_BUILT = {}


def _build():
    if "nc" in _BUILT:
        return _BUILT["nc"]
    nc = bacc.Bacc()

    xT = nc.dram_tensor("xT", [D, BT], f32r, kind="ExternalInput")
    wqT = nc.dram_tensor("wqT", [D, QF], f32r, kind="ExternalInput")
    wkvT = nc.dram_tensor("wkvT", [D, P], f32r, kind="ExternalInput")
    woT = nc.dram_tensor("woT", [QF, D], f32r, kind="ExternalInput")
    bq_d = nc.dram_tensor("bq", [QF, 1], f32, kind="ExternalInput")
    bqn_d = nc.dram_tensor("bqn", [QF, 1], f32, kind="ExternalInput")
    bkv_d = nc.dram_tensor("bkv", [P, 1], f32, kind="ExternalInput")
    bkvn_d = nc.dram_tensor("bkvn", [P, 1], f32, kind="ExternalInput")
    bo_d = nc.dram_tensor("bo", [D, 1], f32, kind="ExternalInput")
    cosq_d = nc.dram_tensor("cosq", [KF, T], f32, kind="ExternalInput")
    sinq_d = nc.dram_tensor("sinq", [KF, T], f32, kind="ExternalInput")
    cosk_d = nc.dram_tensor("cosk", [KF, T], f32, kind="ExternalInput")
    sink_d = nc.dram_tensor("sink", [KF, T], f32, kind="ExternalInput")
    ones_d = nc.dram_tensor("ones", [P, KF], f32r, kind="ExternalInput")
    yT = nc.dram_tensor("yT", [D, BT], f32, kind="ExternalOutput")

    with tile.TileContext(nc) as tc:
        with (
            tc.tile_pool(name="const", bufs=1) as cpool,
            tc.tile_pool(name="xs", bufs=4) as xpool,
            tc.tile_pool(name="work", bufs=2) as wpool,
            tc.tile_pool(name="work2", bufs=2) as wpool2,
            tc.tile_pool(name="es", bufs=3) as epool,
            tc.tile_pool(name="ps", bufs=6, space="PSUM") as ppool,
        ):
            # ---- constants / weights ----
            wq_sb = cpool.tile([P, KT, QF], f32r)
            wkv_sb = cpool.tile([P, KT, P], f32r)
            wo_sb = cpool.tile([P, MB, D], f32r)
            nc.sync.dma_start(
                out=wq_sb[:], in_=wqT[:, :].rearrange("(kt p) m -> p kt m", p=P))
            nc.sync.dma_start(
                out=wkv_sb[:], in_=wkvT[:, :].rearrange("(kt p) m -> p kt m", p=P))
            nc.sync.dma_start(
                out=wo_sb[:], in_=woT[:, :].rearrange("(k2 p) d -> p k2 d", p=P))
            cq_sb = cpool.tile([P, T], f32)
            sq_sb = cpool.tile([P, T], f32)
            ck_sb = cpool.tile([KF, T], f32)
            sk_sb = cpool.tile([KF, T], f32)
            for half in range(2):
                nc.sync.dma_start(out=cq_sb[half * KF:(half + 1) * KF, :],
                                  in_=cosq_d[:, :])
                nc.sync.dma_start(out=sq_sb[half * KF:(half + 1) * KF, :],
                                  in_=sinq_d[:, :])
            nc.sync.dma_start(out=ck_sb[:], in_=cosk_d[:, :])
            nc.sync.dma_start(out=sk_sb[:], in_=sink_d[:, :])
            bq_sb = cpool.tile([P, MB, 1], f32)
            bqn_sb = cpool.tile([P, MB, 1], f32)
            nc.sync.dma_start(
                out=bq_sb[:], in_=bq_d[:, :].rearrange("(mb p) o -> p mb o", p=P))
            nc.sync.dma_start(
                out=bqn_sb[:], in_=bqn_d[:, :].rearrange("(mb p) o -> p mb o", p=P))
            bkv_sb = cpool.tile([P, 1], f32)
            bkvn_sb = cpool.tile([P, 1], f32)
            nc.sync.dma_start(out=bkv_sb[:], in_=bkv_d[:, :])
            nc.sync.dma_start(out=bkvn_sb[:], in_=bkvn_d[:, :])
            bo_sb = cpool.tile([P, KT, 1], f32)
            nc.sync.dma_start(
                out=bo_sb[:], in_=bo_d[:, :].rearrange("(kt p) o -> p kt o", p=P))
            ident = cpool.tile([P, P], f32)
            make_identity(nc, ident[:])
            ones_sb = cpool.tile([1, KF], f32r)
            nc.sync.dma_start(out=ones_sb[:], in_=ones_d[0:1, 0:KF])

            # per-batch resident activations
            qT_sb, kT_sb, vaug_sb, aT_sb = [], [], [], []
            for b in range(B):
                qT_sb.append(cpool.tile([P, MB, T], f32r, name=f"qT{b}"))
                # kT holds K twice: rows 0:64 and 64:128 are identical, so
                # odd q-heads (stored at partition base 64) can matmul against
                # a stationary with a matching base partition.
                kT_sb.append(cpool.tile([P, T], f32r, name=f"kT{b}"))
                vaug_sb.append(cpool.tile([P, TBP, HD + 1], f32r, name=f"vaug{b}"))
                aT_sb.append(cpool.tile([P, MB, T], f32r, name=f"aT{b}"))
                nc.sync.dma_start(
                    out=vaug_sb[b][:, :, HD:HD + 1],
                    in_=ones_d[:, 0:TBP].rearrange("p (t o) -> p t o", o=1))

            for b in range(B):
                # ---- phase B: projections + RoPE for this batch ----
                for lc in range(QCH):          # 512-token chunks within batch
                    poff = lc * NCHUNK
                    col = b * T + poff          # column in xT/yT token space
                    ps_q0 = ppool.tile([P, NCHUNK], f32, tag="ps", name="ps_q0")
                    ps_q1 = ppool.tile([P, NCHUNK], f32, tag="ps", name="ps_q1")
                    ps_kv = ppool.tile([P, NCHUNK], f32, tag="ps", name="ps_kv")
                    for kt in range(KT):
                        x_sb = xpool.tile([P, NCHUNK], f32r, tag="x", name="x_sb")
                        nc.sync.dma_start(
                            out=x_sb[:],
                            in_=xT[kt * P:(kt + 1) * P, col:col + NCHUNK])
                        st, sp = kt == 0, kt == KT - 1
                        xr = x_sb[:]
                        nc.tensor.matmul(ps_q0[:], wq_sb[:, kt, 0:P],
                                         xr, start=st, stop=sp, skip_group_check=True)
                        nc.tensor.matmul(ps_q1[:], wq_sb[:, kt, P:QF],
                                         xr, start=st, stop=sp, skip_group_check=True)
                        nc.tensor.matmul(ps_kv[:], wkv_sb[:, kt, :],
                                         xr, start=st, stop=sp, skip_group_check=True)
                    # RoPE on Q blocks -> qT_sb   (cos/sin tables pre-scaled by 1/8)
                    for mb in range(MB):
                        ps_q = ps_q0 if mb == 0 else ps_q1
                        rot = wpool.tile([P, NCHUNK], f32, tag="rot", name="rot")
                        for g in range(2):
                            r0 = g * 64
                            nc.scalar.activation(
                                rot[r0:r0 + 32, :], ps_q[r0 + 32:r0 + 64, :],
                                AF.Identity, bias=bqn_sb[r0 + 32:r0 + 64, mb, :],
                                scale=-1.0)
                            nc.scalar.activation(
                                rot[r0 + 32:r0 + 64, :], ps_q[r0:r0 + 32, :],
                                AF.Identity, bias=bq_sb[r0:r0 + 32, mb, :],
                                scale=1.0)
                        qcos = wpool.tile([P, NCHUNK], f32, tag="qcos", name="qcos")
                        nc.vector.scalar_tensor_tensor(
                            qcos[:], ps_q[:], bq_sb[:, mb, :],
                            cq_sb[:, poff:poff + NCHUNK], OP.add, OP.mult)
                        nc.vector.tensor_mul(rot[:], rot[:],
                                             sq_sb[:, poff:poff + NCHUNK])
                        nc.vector.tensor_add(
                            qT_sb[b][:, mb, poff:poff + NCHUNK], qcos[:], rot[:])
                    # RoPE on K rows (0:64 of kv)
                    rotk = wpool2.tile([KF, NCHUNK], f32, tag="rotk", name="rotk")
                    nc.scalar.activation(rotk[0:32, :], ps_kv[32:64, :], AF.Identity,
                                         bias=bkvn_sb[32:64, :], scale=-1.0)
                    nc.scalar.activation(rotk[32:64, :], ps_kv[0:32, :], AF.Identity,
                                         bias=bkv_sb[0:32, :], scale=1.0)
                    kcos = wpool2.tile([KF, NCHUNK], f32, tag="kcos", name="kcos")
                    nc.vector.scalar_tensor_tensor(
                        kcos[:], ps_kv[0:KF, :], bkv_sb[0:KF, :],
                        ck_sb[:, poff:poff + NCHUNK], OP.add, OP.mult)
                    nc.vector.tensor_mul(rotk[:], rotk[:],
                                         sk_sb[:, poff:poff + NCHUNK])
                    nc.vector.tensor_add(kT_sb[b][0:KF, poff:poff + NCHUNK],
                                         kcos[:], rotk[:])
                    nc.vector.tensor_add(kT_sb[b][KF:P, poff:poff + NCHUNK],
                                         kcos[:], rotk[:])
                    # V rows (64:128 of kv): bias, then PE-transpose into (k, hd)
                    vt = wpool2.tile([KF, NCHUNK], f32, tag="vt", name="vt")
                    nc.scalar.activation(vt[:], ps_kv[KF:P, :], AF.Identity,
                                         bias=bkv_sb[KF:P, :], scale=1.0)
                    for j in range(NCHUNK // P):
                        ps_vt = ppool.tile([P, HD], f32, tag="ps", name="ps_vt")
                        nc.tensor.transpose(ps_vt[:], vt[:, j * P:(j + 1) * P],
                                            ident[0:KF, 0:KF])
                        slot = lc * (NCHUNK // P) + j
                        nc.vector.tensor_copy(vaug_sb[b][:, slot, 0:HD], ps_vt[:])

                # ---- phase C: attention for this batch ----
                for qc in range(QCH):
                    qoff = qc * NCHUNK
                    for h in range(NH):
                        mb, hr = h // 2, (h % 2) * 64
                        q_mv = qT_sb[b][hr:hr + 64, mb, qoff:qoff + NCHUNK]
                        ps_av = ppool.tile([HD + 1, NCHUNK], f32, tag="ps",
                                           name="ps_av")
                        for kt in range(TBP):
                            ps_s = ppool.tile([P, NCHUNK], f32, tag="ps", name="ps_s")
                            nc.tensor.matmul(
                                ps_s[:],
                                kT_sb[b][hr:hr + 64, kt * P:(kt + 1) * P],
                                q_mv, start=True, stop=True,
                                skip_group_check=True)
                            es = epool.tile([P, NCHUNK], f32r, tag="es", name="es")
                            nc.scalar.activation(es[:], ps_s[:], AF.Exp)
                            nc.tensor.matmul(
                                ps_av[:], vaug_sb[b][:, kt, :],
                                es[:], start=(kt == 0),
                                stop=(kt == TBP - 1), skip_group_check=True)
                        rcp = wpool2.tile([1, NCHUNK], f32r, tag="rcp", name="rcp")
                        with nc.allow_low_precision(
                                reason="f32r softmax denom; ~16 mantissa bits is plenty"):
                            nc.vector.reciprocal(rcp[:], ps_av[HD:HD + 1, :])
                        ps_bc = ppool.tile([HD, NCHUNK], f32, tag="ps", name="ps_bc")
                        nc.tensor.matmul(ps_bc[:], ones_sb[:],
                                         rcp[:], start=True, stop=True,
                                         skip_group_check=True)
                        bc_sb = wpool2.tile([HD, NCHUNK], f32, tag="bc", name="bc_sb")
                        nc.scalar.activation(bc_sb[:], ps_bc[:], AF.Copy)
                        nc.vector.tensor_mul(
                            aT_sb[b][hr:hr + 64, mb, qoff:qoff + NCHUNK],
                            ps_av[0:HD, :], bc_sb[:])

                # ---- phase D: partial output projection for this batch ----
                for qc in range(QCH):
                    qoff = qc * NCHUNK
                    col = b * T + qoff
                    for mo in range(KT):
                        ps_y = ppool.tile([P, NCHUNK], f32, tag="ps", name="ps_y")
                        for k2 in range(MB):
                            nc.tensor.matmul(
                                ps_y[:], wo_sb[:, k2, mo * P:(mo + 1) * P],
                                aT_sb[b][:, k2, qoff:qoff + NCHUNK],
                                start=(k2 == 0), stop=(k2 == MB - 1),
                                skip_group_check=True)
                        yst = wpool.tile([P, NCHUNK], f32, tag="yst", name="yst")
                        nc.scalar.activation(yst[:], ps_y[:], AF.Identity,
                                             bias=bo_sb[:, mo, :], scale=1.0)
                        nc.sync.dma_start(
                            out=yT[mo * P:(mo + 1) * P, col:col + NCHUNK],
                            in_=yst[:])

    nc.finalize()
    _BUILT["nc"] = nc
    return nc


def _rope_tables():
    invf = 1.0 / (ROPE_BASE ** (np.arange(0, HD, 2, dtype=np.float64) / HD))  # (32,)
    ang = np.arange(T, dtype=np.float64)[None, :] * invf[:, None]             # (32, T)
    cos64 = np.concatenate([np.cos(ang), np.cos(ang)], axis=0)                # (64, T)
    sin64 = np.concatenate([np.sin(ang), np.sin(ang)], axis=0)
    return cos64.astype(np.float32), sin64.astype(np.float32)


def _in_maps(x, Wq, bq, Wk, bk, Wv, bv, Wo, bo):
    x = np.asarray(x, np.float32)
    Wq, Wk, Wv, Wo = (np.asarray(a, np.float32) for a in (Wq, Wk, Wv, Wo))
    bq, bk, bv, bo = (np.asarray(a, np.float32) for a in (bq, bk, bv, bo))
    xT = np.ascontiguousarray(x.transpose(2, 0, 1).reshape(D, BT))
    cos64, sin64 = _rope_tables()
    cosq = np.ascontiguousarray(cos64 * SCALE)
    sinq = np.ascontiguousarray(sin64 * SCALE)
    maps = []
    for c in range(8):
        qs = slice(c * QF, (c + 1) * QF)
        ks = slice(c * KF, (c + 1) * KF)
        bq_c = bq[qs].reshape(QF, 1)
        bkv_c = np.concatenate([bk[ks], bv[ks]]).reshape(P, 1)
        bo_c = (bo if c == 0 else np.zeros_like(bo)).reshape(D, 1)
        maps.append({
            "xT": xT,
            "wqT": np.ascontiguousarray(Wq[qs, :].T),
            "wkvT": np.ascontiguousarray(
                np.concatenate([Wk[ks, :], Wv[ks, :]], axis=0).T),
            "woT": np.ascontiguousarray(Wo[:, qs].T),
            "bq": np.ascontiguousarray(bq_c),
            "bqn": np.ascontiguousarray(-bq_c),
            "bkv": np.ascontiguousarray(bkv_c),
            "bkvn": np.ascontiguousarray(-bkv_c),
            "bo": np.ascontiguousarray(bo_c),
            "ones": np.ones((P, KF), np.float32),
            "cosq": cosq, "sinq": sinq,
            "cosk": cos64, "sink": sin64,
        })
    return maps


def _run(in_maps, **kw):
    nc = _build()
    return run_bass_kernel_spmd(nc, in_maps, core_ids=list(range(8)), **kw)


def kernel(x, Wq, bq, Wk, bk, Wv, bv, Wo, bo):
    res = _run(_in_maps(x, Wq, bq, Wk, bk, Wv, bv, Wo, bo))
    y = np.zeros((D, BT), np.float64)
    for r in res.results:
        y += r["yT"].astype(np.float64)
    return np.ascontiguousarray(y.T.reshape(B, T, D)).astype(np.float32)



# revision 9
# speedup vs baseline: 14.0681x; 14.0681x over previous
"""GQA attention (B=2,T=2048,D=2048, HQ=32, HKV=8, RoPE, full softmax) on 8 trn2 cores.

Sharding: one KV head (+ its 4 Q heads) per core (tensor parallel over q-head
groups); each core computes its partial W_o product.

The axon tunnel (~30MB/s) dominates wall time, so host<->device bytes are
minimized:
  - x uploads fp16, token-sharded (2MB/core), and is all-gathered on device
    by an XLA collective program chained in front of the bass kernel.
  - weights upload fp16 per-core slices (~2.5MB/core total).
  - RoPE cos/sin tables ride inside the NEFF as Const tensors (zero per-call
    transfer); the softmax ones-column comes from an on-device memset.
  - the eight partial W_o products are psum_scatter-reduced on device and the
    final (BT, D) result is pulled back once, fp16.

On-device layouts are transposed (features-on-partitions, tokens-on-free) so
every matmul streams a >=256-wide moving dim in fp32r (1 cycle/row); fp16
inputs are widened to fp32r on load. Softmax denominator comes for free from
a ones-column appended to V.
"""

import os
import sys

import numpy as np

for _p in ("/opt/trn_rl_repo", "/root/.axon_site/_ro/trn_rl_repo"):
    if os.path.isdir(_p) and _p not in sys.path:
        sys.path.append(_p)

os.environ.setdefault("JAX_PLATFORMS", "axon,cpu")

import jax
import jax.numpy as jnp
from jax.sharding import Mesh, NamedSharding, PartitionSpec
from jax.experimental.shard_map import shard_map

import concourse.bacc as bacc
import concourse.bass as bass
import concourse.mybir as mybir
import concourse.tile as tile
from concourse import bass2jax
from concourse.masks import make_identity

B, T, D = 2, 2048, 2048
HQ, HKV, HD = 32, 8, 64
NH = HQ // HKV        # 4 q heads per core
QF = NH * HD          # 256 q features per core
KF = HD               # 64 k (or v) features per core
BT = B * T            # 4096
P = 128
NCHUNK = 512          # token chunk (moving dim)
NCH = BT // NCHUNK    # 8
KT = D // P           # 16 contraction tiles over D
TBP = T // P          # 16 key tiles per batch
QCH = T // NCHUNK     # 4 q chunks per batch
MB = QF // P          # 2 q-feature blocks
ROPE_BASE = 10000.0
SCALE = 1.0 / 8.0     # 1/sqrt(HD)
N = 8                 # cores

f16 = mybir.dt.float16
f32 = mybir.dt.float32
f32r = mybir.dt.float32r
AF = mybir.ActivationFunctionType
OP = mybir.AluOpType

_STATE = {}


def _inline_const(nc, data, name, dtype):
    """inline_tensor with an explicit BIR dtype (e.g. f32r from np f32 data)."""
    import base64
    import io
    data = np.ascontiguousarray(data)
    mls = nc._tensor(name, list(data.shape), dtype, kind="Const", type="DRAM")
    buf = io.BytesIO()
    np.save(buf, data, allow_pickle=False)
    mls.file = f"{name}.npy"
    mls.ant_data = base64.standard_b64encode(buf.getvalue()).decode()
    return bass.DRamTensorHandle(name, list(data.shape), dtype)


def _rope_tables():
    invf = 1.0 / (ROPE_BASE ** (np.arange(0, HD, 2, dtype=np.float64) / HD))  # (32,)
    ang = np.arange(T, dtype=np.float64)[None, :] * invf[:, None]             # (32, T)
    cos64 = np.concatenate([np.cos(ang), np.cos(ang)], axis=0)                # (64, T)
    sin64 = np.concatenate([np.sin(ang), np.sin(ang)], axis=0)
    return cos64.astype(np.float32), sin64.astype(np.float32)


def _build():
    nc = bacc.Bacc()

    xT = nc.dram_tensor("xT", [D, BT], f16, kind="ExternalInput")
    wqT = nc.dram_tensor("wqT", [D, QF], f16, kind="ExternalInput")
    wkvT = nc.dram_tensor("wkvT", [D, P], f16, kind="ExternalInput")
    woT = nc.dram_tensor("woT", [QF, D], f16, kind="ExternalInput")
    bq_d = nc.dram_tensor("bq", [QF, 1], f32, kind="ExternalInput")
    bqn_d = nc.dram_tensor("bqn", [QF, 1], f32, kind="ExternalInput")
    bkv_d = nc.dram_tensor("bkv", [P, 1], f32, kind="ExternalInput")
    bkvn_d = nc.dram_tensor("bkvn", [P, 1], f32, kind="ExternalInput")
    bo_d = nc.dram_tensor("bo", [D, 1], f32, kind="ExternalInput")
    yT = nc.dram_tensor("yT", [D, BT], f32, kind="ExternalOutput")

    cos64, sin64 = _rope_tables()
    cq_c = nc.inline_tensor(
        np.concatenate([cos64 * SCALE, cos64 * SCALE], axis=0), name="cq128")
    sq_c = nc.inline_tensor(
        np.concatenate([sin64 * SCALE, sin64 * SCALE], axis=0), name="sq128")
    ck_c = nc.inline_tensor(cos64, name="ck64")
    sk_c = nc.inline_tensor(sin64, name="sk64")
    ones_c = _inline_const(nc, np.ones((P, KF), np.float32), "ones128", f32r)

    with tile.TileContext(nc) as tc:
        with (
            tc.tile_pool(name="const", bufs=1) as cpool,
            tc.tile_pool(name="xs", bufs=3) as xpool,
            tc.tile_pool(name="xh", bufs=2) as hpool,
            tc.tile_pool(name="work", bufs=2) as wpool,
            tc.tile_pool(name="work2", bufs=2) as wpool2,
            tc.tile_pool(name="es", bufs=2) as epool,
            tc.tile_pool(name="ps", bufs=6, space="PSUM") as ppool,
        ):
            # ---- weights: fp16 -> fp32r, streamed through the x staging
            # tiles in (P, <=512) chunks so no extra SBUF is reserved ----
            wq_sb = cpool.tile([P, KT, QF], f32r)
            wkv_sb = cpool.tile([P, KT, P], f32r)
            wo_sb = cpool.tile([P, MB, D], f32r)
            for kt in range(KT):
                wh = hpool.tile([P, NCHUNK], f16, tag="xh", name="x_h")
                nc.sync.dma_start(out=wh[:, 0:QF],
                                  in_=wqT[kt * P:(kt + 1) * P, :])
                nc.sync.dma_start(out=wh[:, QF:QF + P],
                                  in_=wkvT[kt * P:(kt + 1) * P, :])
                nc.vector.tensor_copy(wq_sb[:, kt, :], wh[:, 0:QF])
                nc.vector.tensor_copy(wkv_sb[:, kt, :], wh[:, QF:QF + P])
            for k2 in range(MB):
                for j in range(D // NCHUNK):
                    wh = hpool.tile([P, NCHUNK], f16, tag="xh", name="x_h")
                    nc.sync.dma_start(
                        out=wh[:],
                        in_=woT[k2 * P:(k2 + 1) * P, j * NCHUNK:(j + 1) * NCHUNK])
                    nc.vector.tensor_copy(wo_sb[:, k2, j * NCHUNK:(j + 1) * NCHUNK],
                                          wh[:])

            # ---- constants ----
            cq_sb = cpool.tile([P, T], f32)
            sq_sb = cpool.tile([P, T], f32)
            ck_sb = cpool.tile([KF, T], f32)
            sk_sb = cpool.tile([KF, T], f32)
            nc.sync.dma_start(out=cq_sb[:], in_=cq_c[:, :])
            nc.sync.dma_start(out=sq_sb[:], in_=sq_c[:, :])
            nc.sync.dma_start(out=ck_sb[:], in_=ck_c[:, :])
            nc.sync.dma_start(out=sk_sb[:], in_=sk_c[:, :])
            bq_sb = cpool.tile([P, MB, 1], f32)
            bqn_sb = cpool.tile([P, MB, 1], f32)
            nc.sync.dma_start(
                out=bq_sb[:], in_=bq_d[:, :].rearrange("(mb p) o -> p mb o", p=P))
            nc.sync.dma_start(
                out=bqn_sb[:], in_=bqn_d[:, :].rearrange("(mb p) o -> p mb o", p=P))
            bkv_sb = cpool.tile([P, 1], f32)
            bkvn_sb = cpool.tile([P, 1], f32)
            nc.sync.dma_start(out=bkv_sb[:], in_=bkv_d[:, :])
            nc.sync.dma_start(out=bkvn_sb[:], in_=bkvn_d[:, :])
            bo_sb = cpool.tile([P, KT, 1], f32)
            nc.sync.dma_start(
                out=bo_sb[:], in_=bo_d[:, :].rearrange("(kt p) o -> p kt o", p=P))
            ident = cpool.tile([P, P], f32)
            make_identity(nc, ident[:])
            ones_sb = cpool.tile([1, KF], f32r)
            nc.sync.dma_start(out=ones_sb[:], in_=ones_c[0:1, 0:KF])

            # per-batch resident activations
            qT_sb, kT_sb, vaug_sb, aT_sb = [], [], [], []
            for b in range(B):
                qT_sb.append(cpool.tile([P, MB, T], f32r, name=f"qT{b}"))
                # kT holds K twice: rows 0:64 and 64:128 are identical, so
                # odd q-heads (stored at partition base 64) can matmul against
                # a stationary with a matching base partition.
                kT_sb.append(cpool.tile([P, T], f32r, name=f"kT{b}"))
                vaug_sb.append(cpool.tile([P, TBP, HD + 1], f32r, name=f"vaug{b}"))
                aT_sb.append(cpool.tile([P, MB, T], f32r, name=f"aT{b}"))
                nc.sync.dma_start(
                    out=vaug_sb[b][:, :, HD:HD + 1],
                    in_=ones_c[:, 0:TBP].rearrange("p (t o) -> p t o", o=1))

            for b in range(B):
                # ---- phase B: projections + RoPE for this batch ----
                for lc in range(QCH):          # 512-token chunks within batch
                    poff = lc * NCHUNK
                    col = b * T + poff          # column in xT/yT token space
                    ps_q0 = ppool.tile([P, NCHUNK], f32, tag="ps", name="ps_q0")
                    ps_q1 = ppool.tile([P, NCHUNK], f32, tag="ps", name="ps_q1")
                    ps_kv = ppool.tile([P, NCHUNK], f32, tag="ps", name="ps_kv")
                    for kt in range(KT):
                        x_h = hpool.tile([P, NCHUNK], f16, tag="xh", name="x_h")
                        nc.sync.dma_start(
                            out=x_h[:],
                            in_=xT[kt * P:(kt + 1) * P, col:col + NCHUNK])
                        x_sb = xpool.tile([P, NCHUNK], f32r, tag="x", name="x_sb")
                        nc.scalar.activation(x_sb[:], x_h[:], AF.Copy)
                        st, sp = kt == 0, kt == KT - 1
                        xr = x_sb[:]
                        nc.tensor.matmul(ps_q0[:], wq_sb[:, kt, 0:P],
                                         xr, start=st, stop=sp, skip_group_check=True)
                        nc.tensor.matmul(ps_q1[:], wq_sb[:, kt, P:QF],
                                         xr, start=st, stop=sp, skip_group_check=True)
                        nc.tensor.matmul(ps_kv[:], wkv_sb[:, kt, :],
                                         xr, start=st, stop=sp, skip_group_check=True)
                    # RoPE on Q blocks -> qT_sb   (cos/sin tables pre-scaled by 1/8)
                    for mb in range(MB):
                        ps_q = ps_q0 if mb == 0 else ps_q1
                        rot = wpool.tile([P, NCHUNK], f32, tag="rot", name="rot")
                        for g in range(2):
                            r0 = g * 64
                            nc.scalar.activation(
                                rot[r0:r0 + 32, :], ps_q[r0 + 32:r0 + 64, :],
                                AF.Identity, bias=bqn_sb[r0 + 32:r0 + 64, mb, :],
                                scale=-1.0)
                            nc.scalar.activation(
                                rot[r0 + 32:r0 + 64, :], ps_q[r0:r0 + 32, :],
                                AF.Identity, bias=bq_sb[r0:r0 + 32, mb, :],
                                scale=1.0)
                        qcos = wpool.tile([P, NCHUNK], f32, tag="qcos", name="qcos")
                        nc.vector.scalar_tensor_tensor(
                            qcos[:], ps_q[:], bq_sb[:, mb, :],
                            cq_sb[:, poff:poff + NCHUNK], OP.add, OP.mult)
                        nc.vector.tensor_mul(rot[:], rot[:],
                                             sq_sb[:, poff:poff + NCHUNK])
                        nc.vector.tensor_add(
                            qT_sb[b][:, mb, poff:poff + NCHUNK], qcos[:], rot[:])
                    # RoPE on K rows (0:64 of kv)
                    rotk = wpool2.tile([KF, NCHUNK], f32, tag="rotk", name="rotk")
                    nc.scalar.activation(rotk[0:32, :], ps_kv[32:64, :], AF.Identity,
                                         bias=bkvn_sb[32:64, :], scale=-1.0)
                    nc.scalar.activation(rotk[32:64, :], ps_kv[0:32, :], AF.Identity,
                                         bias=bkv_sb[0:32, :], scale=1.0)
                    kcos = wpool2.tile([KF, NCHUNK], f32, tag="kcos", name="kcos")
                    nc.vector.scalar_tensor_tensor(
                        kcos[:], ps_kv[0:KF, :], bkv_sb[0:KF, :],
                        ck_sb[:, poff:poff + NCHUNK], OP.add, OP.mult)
                    nc.vector.tensor_mul(rotk[:], rotk[:],
                                         sk_sb[:, poff:poff + NCHUNK])
                    nc.vector.tensor_add(kT_sb[b][0:KF, poff:poff + NCHUNK],
                                         kcos[:], rotk[:])
                    nc.vector.tensor_add(kT_sb[b][KF:P, poff:poff + NCHUNK],
                                         kcos[:], rotk[:])
                    # V rows (64:128 of kv): bias, then PE-transpose into (k, hd)
                    vt = wpool2.tile([KF, NCHUNK], f32, tag="vt", name="vt")
                    nc.scalar.activation(vt[:], ps_kv[KF:P, :], AF.Identity,
                                         bias=bkv_sb[KF:P, :], scale=1.0)
                    for j in range(NCHUNK // P):
                        ps_vt = ppool.tile([P, HD], f32, tag="ps", name="ps_vt")
                        nc.tensor.transpose(ps_vt[:], vt[:, j * P:(j + 1) * P],
                                            ident[0:KF, 0:KF])
                        slot = lc * (NCHUNK // P) + j
                        nc.vector.tensor_copy(vaug_sb[b][:, slot, 0:HD], ps_vt[:])

                # ---- phase C: attention for this batch ----
                for qc in range(QCH):
                    qoff = qc * NCHUNK
                    for h in range(NH):
                        mb, hr = h // 2, (h % 2) * 64
                        q_mv = qT_sb[b][hr:hr + 64, mb, qoff:qoff + NCHUNK]
                        ps_av = ppool.tile([HD + 1, NCHUNK], f32, tag="ps",
                                           name="ps_av")
                        for kt in range(TBP):
                            ps_s = ppool.tile([P, NCHUNK], f32, tag="ps", name="ps_s")
                            nc.tensor.matmul(
                                ps_s[:],
                                kT_sb[b][hr:hr + 64, kt * P:(kt + 1) * P],
                                q_mv, start=True, stop=True,
                                skip_group_check=True)
                            es = epool.tile([P, NCHUNK], f32r, tag="es", name="es")
                            nc.scalar.activation(es[:], ps_s[:], AF.Exp)
                            nc.tensor.matmul(
                                ps_av[:], vaug_sb[b][:, kt, :],
                                es[:], start=(kt == 0),
                                stop=(kt == TBP - 1), skip_group_check=True)
                        rcp = wpool2.tile([1, NCHUNK], f32r, tag="rcp", name="rcp")
                        with nc.allow_low_precision(
                                reason="f32r softmax denom; ~16 mantissa bits is plenty"):
                            nc.vector.reciprocal(rcp[:], ps_av[HD:HD + 1, :])
                        ps_bc = ppool.tile([HD, NCHUNK], f32, tag="ps", name="ps_bc")
                        nc.tensor.matmul(ps_bc[:], ones_sb[:],
                                         rcp[:], start=True, stop=True,
                                         skip_group_check=True)
                        bc_sb = wpool2.tile([HD, NCHUNK], f32, tag="bc", name="bc_sb")
                        nc.scalar.activation(bc_sb[:], ps_bc[:], AF.Copy)
                        nc.vector.tensor_mul(
                            aT_sb[b][hr:hr + 64, mb, qoff:qoff + NCHUNK],
                            ps_av[0:HD, :], bc_sb[:])

                # ---- phase D: partial output projection for this batch ----
                for qc in range(QCH):
                    qoff = qc * NCHUNK
                    col = b * T + qoff
                    for mo in range(KT):
                        ps_y = ppool.tile([P, NCHUNK], f32, tag="ps", name="ps_y")
                        for k2 in range(MB):
                            nc.tensor.matmul(
                                ps_y[:], wo_sb[:, k2, mo * P:(mo + 1) * P],
                                aT_sb[b][:, k2, qoff:qoff + NCHUNK],
                                start=(k2 == 0), stop=(k2 == MB - 1),
                                skip_group_check=True)
                        yst = wpool.tile([P, NCHUNK], f32, tag="yst", name="yst")
                        nc.scalar.activation(yst[:], ps_y[:], AF.Identity,
                                             bias=bo_sb[:, mo, :], scale=1.0)
                        nc.sync.dma_start(
                            out=yT[mo * P:(mo + 1) * P, col:col + NCHUNK],
                            in_=yst[:])

    nc.finalize()
    return nc


def _get_state():
    if _STATE:
        return _STATE
    nc = _build()
    bass2jax.install_neuronx_cc_hook()

    partition_name = nc.partition_id_tensor.name if nc.partition_id_tensor else None
    in_names, out_names, out_avals = [], [], []
    for alloc in nc.m.functions[0].allocations:
        if not isinstance(alloc, mybir.MemoryLocationSet):
            continue
        name = alloc.memorylocations[0].name
        if alloc.kind == "ExternalInput":
            if name != partition_name:
                in_names.append(name)
        elif alloc.kind == "ExternalOutput":
            out_names.append(name)
            out_avals.append(jax.core.ShapedArray(
                tuple(alloc.tensor_shape), mybir.dt.np(alloc.dtype)))
    n_params = len(in_names)
    n_outs = len(out_avals)
    in_names_all = in_names + out_names
    if partition_name is not None:
        in_names_all.append(partition_name)

    devices = jax.devices()[:N]
    mesh = Mesh(np.asarray(devices), ("core",))
    shard0 = NamedSharding(mesh, PartitionSpec("core"))

    def _body(*args):
        operands = list(args)
        if partition_name is not None:
            operands.append(bass2jax.partition_id_tensor())
        outs = bass2jax._bass_exec_p.bind(
            *operands,
            out_avals=tuple(out_avals),
            in_names=tuple(in_names_all),
            out_names=tuple(out_names),
            lowering_input_output_aliases=(),
            sim_require_finite=True,
            sim_require_nnan=True,
            nc=nc,
        )
        return tuple(outs)

    donate = tuple(range(n_params, n_params + n_outs))
    jit_bass = jax.jit(
        shard_map(_body, mesh=mesh,
                  in_specs=(PartitionSpec("core"),) * (n_params + n_outs),
                  out_specs=(PartitionSpec("core"),) * n_outs,
                  check_rep=False),
        donate_argnums=donate, keep_unused=True,
    )

    # x: (BT, D) f16 token-sharded -> transpose + all-gather -> per-core full
    # xT (D, BT), stacked to the (N*D, BT) global the bass program expects.
    def _gather(xl):
        return jax.lax.all_gather(jnp.transpose(xl), "core", axis=1, tiled=True)

    jit_gather = jax.jit(
        shard_map(_gather, mesh=mesh,
                  in_specs=PartitionSpec("core", None),
                  out_specs=PartitionSpec("core"), check_rep=False))

    # yT partials (N*D, BT) -> on-device sum, each core keeps a D/N row
    # slice, transposed so the pulled global is y_flat (BT, D), fp16.
    def _reduce(yl):
        ys = jax.lax.psum_scatter(yl, "core", scatter_dimension=0, tiled=True)
        return jnp.transpose(ys).astype(jnp.float16)

    jit_reduce = jax.jit(
        shard_map(_reduce, mesh=mesh,
                  in_specs=PartitionSpec("core"),
                  out_specs=PartitionSpec(None, "core"), check_rep=False))

    jit_zeros = jax.jit(
        lambda: jnp.zeros((N * D, BT), jnp.float32), out_shardings=shard0)

    _STATE.update(
        nc=nc, mesh=mesh, shard0=shard0, in_names=in_names,
        jit_bass=jit_bass, jit_gather=jit_gather, jit_reduce=jit_reduce,
        jit_zeros=jit_zeros,
    )
    return _STATE


def _host_prep(x, Wq, bq, Wk, bk, Wv, bv, Wo, bo):
    """Per-core fp16 weight slices, concatenated core-major for shard_map."""
    x = np.asarray(x, np.float32)
    Wq, Wk, Wv, Wo = (np.asarray(a, np.float32) for a in (Wq, Wk, Wv, Wo))
    bq, bk, bv, bo = (np.asarray(a, np.float32) for a in (bq, bk, bv, bo))

    x_flat = np.ascontiguousarray(x.reshape(BT, D).astype(np.float16))
    wq_cat = np.empty((N * D, QF), np.float16)
    wkv_cat = np.empty((N * D, P), np.float16)
    wo_cat = np.empty((N * QF, D), np.float16)
    bq_cat = np.empty((N * QF, 1), np.float32)
    bqn_cat = np.empty((N * QF, 1), np.float32)
    bkv_cat = np.empty((N * P, 1), np.float32)
    bkvn_cat = np.empty((N * P, 1), np.float32)
    bo_cat = np.zeros((N * D, 1), np.float32)
    for c in range(N):
        qs = slice(c * QF, (c + 1) * QF)
        ks = slice(c * KF, (c + 1) * KF)
        wq_cat[c * D:(c + 1) * D] = Wq[qs, :].T
        wkv_cat[c * D:(c + 1) * D, 0:KF] = Wk[ks, :].T
        wkv_cat[c * D:(c + 1) * D, KF:P] = Wv[ks, :].T
        wo_cat[c * QF:(c + 1) * QF] = Wo[:, qs].T
        bq_c = bq[qs].reshape(QF, 1)
        bq_cat[c * QF:(c + 1) * QF] = bq_c
        bqn_cat[c * QF:(c + 1) * QF] = -bq_c
        bkv_c = np.concatenate([bk[ks], bv[ks]]).reshape(P, 1)
        bkv_cat[c * P:(c + 1) * P] = bkv_c
        bkvn_cat[c * P:(c + 1) * P] = -bkv_c
    bo_cat[0:D] = bo.reshape(D, 1)
    return x_flat, {
        "wqT": wq_cat, "wkvT": wkv_cat, "woT": wo_cat,
        "bq": bq_cat, "bqn": bqn_cat, "bkv": bkv_cat, "bkvn": bkvn_cat,
        "bo": bo_cat,
    }


def _run(x_flat, cats):
    st = _get_state()
    mesh = st["mesh"]
    xsh = NamedSharding(mesh, PartitionSpec("core", None))
    names = [n for n in st["in_names"] if n != "xT"]
    put = jax.device_put(
        [x_flat] + [cats[n] for n in names],
        [xsh] + [st["shard0"]] * len(names))
    by_name = dict(zip(names, put[1:]))
    by_name["xT"] = st["jit_gather"](put[0])
    args = [by_name[n] for n in st["in_names"]] + [st["jit_zeros"]()]
    (ypart,) = st["jit_bass"](*args)
    return np.asarray(st["jit_reduce"](ypart))     # (BT, D) f16


def kernel(x, Wq, bq, Wk, bk, Wv, bv, Wo, bo):
    x_flat, cats = _host_prep(x, Wq, bq, Wk, bk, Wv, bv, Wo, bo)
    y_flat = _run(x_flat, cats)
    return y_flat.astype(np.float32).reshape(B, T, D)


# revision 13
# speedup vs baseline: 15.4690x; 1.0996x over previous
"""GQA attention (B=2,T=2048,D=2048, HQ=32, HKV=8, RoPE, full softmax) on 8 trn2 cores.

Sharding: one KV head (+ its 4 Q heads) per core (tensor parallel over q-head
groups); each core computes its partial W_o product.

The axon tunnel (~30MB/s) dominates wall time, so host<->device bytes are
minimized:
  - x uploads fp16, token-sharded (2MB/core), and is all-gathered on device
    by an XLA collective program chained in front of the bass kernel.
  - weights upload fp16 per-core slices (~2.5MB/core total).
  - RoPE cos/sin tables ride inside the NEFF as Const tensors (zero per-call
    transfer); the softmax ones-column comes from an on-device memset.
  - the eight partial W_o products are psum_scatter-reduced on device and the
    final (BT, D) result is pulled back once, fp16.

On-device layouts are transposed (features-on-partitions, tokens-on-free) so
every matmul streams a >=256-wide moving dim in fp32r (1 cycle/row); fp16
inputs are widened to fp32r on load. Softmax denominator comes for free from
a ones-column appended to V.
"""

import os
import sys

import numpy as np

for _p in ("/opt/trn_rl_repo", "/root/.axon_site/_ro/trn_rl_repo"):
    if os.path.isdir(_p) and _p not in sys.path:
        sys.path.append(_p)

os.environ.setdefault("JAX_PLATFORMS", "axon,cpu")

import jax
import jax.numpy as jnp
from jax.sharding import Mesh, NamedSharding, PartitionSpec
from jax.experimental.shard_map import shard_map

import concourse.bacc as bacc
import concourse.bass as bass
import concourse.mybir as mybir
import concourse.tile as tile
from concourse import bass2jax
from concourse.masks import make_identity

B, T, D = 2, 2048, 2048
HQ, HKV, HD = 32, 8, 64
NH = HQ // HKV        # 4 q heads per core
QF = NH * HD          # 256 q features per core
KF = HD               # 64 k (or v) features per core
BT = B * T            # 4096
P = 128
NCHUNK = 512          # token chunk (moving dim)
NCH = BT // NCHUNK    # 8
KT = D // P           # 16 contraction tiles over D
TBP = T // P          # 16 key tiles per batch
QCH = T // NCHUNK     # 4 q chunks per batch
MB = QF // P          # 2 q-feature blocks
ROPE_BASE = 10000.0
SCALE = 1.0 / 8.0     # 1/sqrt(HD)
N = 8                 # cores

f16 = mybir.dt.float16
f32 = mybir.dt.float32
f32r = mybir.dt.float32r
AF = mybir.ActivationFunctionType
OP = mybir.AluOpType

_STATE = {}


def _inline_const(nc, data, name, dtype):
    """inline_tensor with an explicit BIR dtype (e.g. f32r from np f32 data)."""
    import base64
    import io
    data = np.ascontiguousarray(data)
    mls = nc._tensor(name, list(data.shape), dtype, kind="Const", type="DRAM")
    buf = io.BytesIO()
    np.save(buf, data, allow_pickle=False)
    mls.file = f"{name}.npy"
    mls.ant_data = base64.standard_b64encode(buf.getvalue()).decode()
    return bass.DRamTensorHandle(name, list(data.shape), dtype)


def _rope_tables():
    invf = 1.0 / (ROPE_BASE ** (np.arange(0, HD, 2, dtype=np.float64) / HD))  # (32,)
    ang = np.arange(T, dtype=np.float64)[None, :] * invf[:, None]             # (32, T)
    cos64 = np.concatenate([np.cos(ang), np.cos(ang)], axis=0)                # (64, T)
    sin64 = np.concatenate([np.sin(ang), np.sin(ang)], axis=0)
    return cos64.astype(np.float32), sin64.astype(np.float32)


def _build():
    nc = bacc.Bacc()

    xT = nc.dram_tensor("xT", [D, BT], f16, kind="ExternalInput")
    wqT = nc.dram_tensor("wqT", [D, QF], f16, kind="ExternalInput")
    wkvT = nc.dram_tensor("wkvT", [D, P], f16, kind="ExternalInput")
    woT = nc.dram_tensor("woT", [QF, D], f16, kind="ExternalInput")
    bq_d = nc.dram_tensor("bq", [QF, 1], f32, kind="ExternalInput")
    bqn_d = nc.dram_tensor("bqn", [QF, 1], f32, kind="ExternalInput")
    bkv_d = nc.dram_tensor("bkv", [P, 1], f32, kind="ExternalInput")
    bkvn_d = nc.dram_tensor("bkvn", [P, 1], f32, kind="ExternalInput")
    bo_d = nc.dram_tensor("bo", [D, 1], f32, kind="ExternalInput")
    yT = nc.dram_tensor("yT", [D, BT], f32, kind="ExternalOutput")

    cos64, sin64 = _rope_tables()
    cq_c = nc.inline_tensor(
        np.concatenate([cos64 * SCALE, cos64 * SCALE], axis=0), name="cq128")
    sq_c = nc.inline_tensor(
        np.concatenate([sin64 * SCALE, sin64 * SCALE], axis=0), name="sq128")
    ck_c = nc.inline_tensor(cos64, name="ck64")
    sk_c = nc.inline_tensor(sin64, name="sk64")
    ones_c = _inline_const(nc, np.ones((P, KF), np.float32), "ones128", f32r)

    with tile.TileContext(nc) as tc:
        with (
            tc.tile_pool(name="const", bufs=1) as cpool,
            tc.tile_pool(name="xs", bufs=3) as xpool,
            tc.tile_pool(name="xh", bufs=2) as hpool,
            tc.tile_pool(name="work", bufs=2) as wpool,
            tc.tile_pool(name="work2", bufs=2) as wpool2,
            tc.tile_pool(name="es", bufs=2) as epool,
            tc.tile_pool(name="ps", bufs=6, space="PSUM") as ppool,
        ):
            # ---- weights: fp16 -> fp32r, streamed through the x staging
            # tiles in (P, <=512) chunks so no extra SBUF is reserved ----
            wq_sb = cpool.tile([P, KT, QF], f32r)
            wkv_sb = cpool.tile([P, KT, P], f32r)
            wo_sb = cpool.tile([P, MB, D], f32r)
            for kt in range(KT):
                wh = hpool.tile([P, NCHUNK], f16, tag="xh", name="x_h")
                nc.sync.dma_start(out=wh[:, 0:QF],
                                  in_=wqT[kt * P:(kt + 1) * P, :])
                nc.sync.dma_start(out=wh[:, QF:QF + P],
                                  in_=wkvT[kt * P:(kt + 1) * P, :])
                nc.vector.tensor_copy(wq_sb[:, kt, :], wh[:, 0:QF])
                nc.vector.tensor_copy(wkv_sb[:, kt, :], wh[:, QF:QF + P])
            for k2 in range(MB):
                for j in range(D // NCHUNK):
                    wh = hpool.tile([P, NCHUNK], f16, tag="xh", name="x_h")
                    nc.sync.dma_start(
                        out=wh[:],
                        in_=woT[k2 * P:(k2 + 1) * P, j * NCHUNK:(j + 1) * NCHUNK])
                    nc.vector.tensor_copy(wo_sb[:, k2, j * NCHUNK:(j + 1) * NCHUNK],
                                          wh[:])

            # ---- constants ----
            cq_sb = cpool.tile([P, T], f32)
            sq_sb = cpool.tile([P, T], f32)
            ck_sb = cpool.tile([KF, T], f32)
            sk_sb = cpool.tile([KF, T], f32)
            nc.sync.dma_start(out=cq_sb[:], in_=cq_c[:, :])
            nc.sync.dma_start(out=sq_sb[:], in_=sq_c[:, :])
            nc.sync.dma_start(out=ck_sb[:], in_=ck_c[:, :])
            nc.sync.dma_start(out=sk_sb[:], in_=sk_c[:, :])
            bq_sb = cpool.tile([P, MB, 1], f32)
            bqn_sb = cpool.tile([P, MB, 1], f32)
            nc.sync.dma_start(
                out=bq_sb[:], in_=bq_d[:, :].rearrange("(mb p) o -> p mb o", p=P))
            nc.sync.dma_start(
                out=bqn_sb[:], in_=bqn_d[:, :].rearrange("(mb p) o -> p mb o", p=P))
            bkv_sb = cpool.tile([P, 1], f32)
            bkvn_sb = cpool.tile([P, 1], f32)
            nc.sync.dma_start(out=bkv_sb[:], in_=bkv_d[:, :])
            nc.sync.dma_start(out=bkvn_sb[:], in_=bkvn_d[:, :])
            bo_sb = cpool.tile([P, KT, 1], f32)
            nc.sync.dma_start(
                out=bo_sb[:], in_=bo_d[:, :].rearrange("(kt p) o -> p kt o", p=P))
            ident = cpool.tile([P, P], f32)
            make_identity(nc, ident[:])
            ones_sb = cpool.tile([1, KF], f32r)
            nc.sync.dma_start(out=ones_sb[:], in_=ones_c[0:1, 0:KF])

            # per-batch resident activations
            qT_sb, kT_sb, vaug_sb, aT_sb = [], [], [], []
            for b in range(B):
                qT_sb.append(cpool.tile([P, MB, T], f32r, name=f"qT{b}"))
                # kT holds K twice: rows 0:64 and 64:128 are identical, so
                # odd q-heads (stored at partition base 64) can matmul against
                # a stationary with a matching base partition.
                kT_sb.append(cpool.tile([P, T], f32r, name=f"kT{b}"))
                vaug_sb.append(cpool.tile([P, TBP, HD + 1], f32r, name=f"vaug{b}"))
                aT_sb.append(cpool.tile([P, MB, T], f32r, name=f"aT{b}"))
                nc.sync.dma_start(
                    out=vaug_sb[b][:, :, HD:HD + 1],
                    in_=ones_c[:, 0:TBP].rearrange("p (t o) -> p t o", o=1))

            for b in range(B):
                # ---- phase B: projections + RoPE for this batch ----
                for lc in range(QCH):          # 512-token chunks within batch
                    poff = lc * NCHUNK
                    col = b * T + poff          # column in xT/yT token space
                    ps_q0 = ppool.tile([P, NCHUNK], f32, tag="ps", name="ps_q0")
                    ps_q1 = ppool.tile([P, NCHUNK], f32, tag="ps", name="ps_q1")
                    ps_kv = ppool.tile([P, NCHUNK], f32, tag="ps", name="ps_kv")
                    for kt in range(KT):
                        x_h = hpool.tile([P, NCHUNK], f16, tag="xh", name="x_h")
                        nc.sync.dma_start(
                            out=x_h[:],
                            in_=xT[kt * P:(kt + 1) * P, col:col + NCHUNK])
                        x_sb = xpool.tile([P, NCHUNK], f32r, tag="x", name="x_sb")
                        nc.scalar.activation(x_sb[:], x_h[:], AF.Copy)
                        st, sp = kt == 0, kt == KT - 1
                        xr = x_sb[:]
                        nc.tensor.matmul(ps_q0[:], wq_sb[:, kt, 0:P],
                                         xr, start=st, stop=sp, skip_group_check=True)
                        nc.tensor.matmul(ps_q1[:], wq_sb[:, kt, P:QF],
                                         xr, start=st, stop=sp, skip_group_check=True)
                        nc.tensor.matmul(ps_kv[:], wkv_sb[:, kt, :],
                                         xr, start=st, stop=sp, skip_group_check=True)
                    # RoPE on Q blocks -> qT_sb   (cos/sin tables pre-scaled by 1/8)
                    for mb in range(MB):
                        ps_q = ps_q0 if mb == 0 else ps_q1
                        rot = wpool.tile([P, NCHUNK], f32, tag="rot", name="rot")
                        for g in range(2):
                            r0 = g * 64
                            nc.scalar.activation(
                                rot[r0:r0 + 32, :], ps_q[r0 + 32:r0 + 64, :],
                                AF.Identity, bias=bqn_sb[r0 + 32:r0 + 64, mb, :],
                                scale=-1.0)
                            nc.scalar.activation(
                                rot[r0 + 32:r0 + 64, :], ps_q[r0:r0 + 32, :],
                                AF.Identity, bias=bq_sb[r0:r0 + 32, mb, :],
                                scale=1.0)
                        qcos = wpool.tile([P, NCHUNK], f32, tag="qcos", name="qcos")
                        nc.vector.scalar_tensor_tensor(
                            qcos[:], ps_q[:], bq_sb[:, mb, :],
                            cq_sb[:, poff:poff + NCHUNK], OP.add, OP.mult)
                        nc.vector.tensor_mul(rot[:], rot[:],
                                             sq_sb[:, poff:poff + NCHUNK])
                        nc.vector.tensor_add(
                            qT_sb[b][:, mb, poff:poff + NCHUNK], qcos[:], rot[:])
                    # RoPE on K rows (0:64 of kv)
                    rotk = wpool2.tile([KF, NCHUNK], f32, tag="rotk", name="rotk")
                    nc.scalar.activation(rotk[0:32, :], ps_kv[32:64, :], AF.Identity,
                                         bias=bkvn_sb[32:64, :], scale=-1.0)
                    nc.scalar.activation(rotk[32:64, :], ps_kv[0:32, :], AF.Identity,
                                         bias=bkv_sb[0:32, :], scale=1.0)
                    kcos = wpool2.tile([KF, NCHUNK], f32, tag="kcos", name="kcos")
                    nc.vector.scalar_tensor_tensor(
                        kcos[:], ps_kv[0:KF, :], bkv_sb[0:KF, :],
                        ck_sb[:, poff:poff + NCHUNK], OP.add, OP.mult)
                    nc.vector.tensor_mul(rotk[:], rotk[:],
                                         sk_sb[:, poff:poff + NCHUNK])
                    nc.vector.tensor_add(kT_sb[b][0:KF, poff:poff + NCHUNK],
                                         kcos[:], rotk[:])
                    nc.vector.tensor_add(kT_sb[b][KF:P, poff:poff + NCHUNK],
                                         kcos[:], rotk[:])
                    # V rows (64:128 of kv): bias, then PE-transpose into (k, hd)
                    vt = wpool2.tile([KF, NCHUNK], f32, tag="vt", name="vt")
                    nc.scalar.activation(vt[:], ps_kv[KF:P, :], AF.Identity,
                                         bias=bkv_sb[KF:P, :], scale=1.0)
                    for j in range(NCHUNK // P):
                        ps_vt = ppool.tile([P, HD], f32, tag="ps", name="ps_vt")
                        nc.tensor.transpose(ps_vt[:], vt[:, j * P:(j + 1) * P],
                                            ident[0:KF, 0:KF])
                        slot = lc * (NCHUNK // P) + j
                        nc.vector.tensor_copy(vaug_sb[b][:, slot, 0:HD], ps_vt[:])

                # ---- phase C: attention for this batch ----
                for qc in range(QCH):
                    qoff = qc * NCHUNK
                    for h in range(NH):
                        mb, hr = h // 2, (h % 2) * 64
                        q_mv = qT_sb[b][hr:hr + 64, mb, qoff:qoff + NCHUNK]
                        ps_av = ppool.tile([HD + 1, NCHUNK], f32, tag="ps",
                                           name="ps_av")
                        for kt in range(TBP):
                            ps_s = ppool.tile([P, NCHUNK], f32, tag="ps", name="ps_s")
                            nc.tensor.matmul(
                                ps_s[:],
                                kT_sb[b][hr:hr + 64, kt * P:(kt + 1) * P],
                                q_mv, start=True, stop=True,
                                skip_group_check=True)
                            es = epool.tile([P, NCHUNK], f32r, tag="es", name="es")
                            nc.scalar.activation(es[:], ps_s[:], AF.Exp)
                            nc.tensor.matmul(
                                ps_av[:], vaug_sb[b][:, kt, :],
                                es[:], start=(kt == 0),
                                stop=(kt == TBP - 1), skip_group_check=True)
                        rcp = wpool2.tile([1, NCHUNK], f32r, tag="rcp", name="rcp")
                        with nc.allow_low_precision(
                                reason="f32r softmax denom; ~16 mantissa bits is plenty"):
                            nc.vector.reciprocal(rcp[:], ps_av[HD:HD + 1, :])
                        ps_bc = ppool.tile([HD, NCHUNK], f32, tag="ps", name="ps_bc")
                        nc.tensor.matmul(ps_bc[:], ones_sb[:],
                                         rcp[:], start=True, stop=True,
                                         skip_group_check=True)
                        bc_sb = wpool2.tile([HD, NCHUNK], f32, tag="bc", name="bc_sb")
                        nc.scalar.activation(bc_sb[:], ps_bc[:], AF.Copy)
                        nc.vector.tensor_mul(
                            aT_sb[b][hr:hr + 64, mb, qoff:qoff + NCHUNK],
                            ps_av[0:HD, :], bc_sb[:])

                # ---- phase D: partial output projection for this batch ----
                for qc in range(QCH):
                    qoff = qc * NCHUNK
                    col = b * T + qoff
                    for mo in range(KT):
                        ps_y = ppool.tile([P, NCHUNK], f32, tag="ps", name="ps_y")
                        for k2 in range(MB):
                            nc.tensor.matmul(
                                ps_y[:], wo_sb[:, k2, mo * P:(mo + 1) * P],
                                aT_sb[b][:, k2, qoff:qoff + NCHUNK],
                                start=(k2 == 0), stop=(k2 == MB - 1),
                                skip_group_check=True)
                        yst = wpool.tile([P, NCHUNK], f32, tag="yst", name="yst")
                        nc.scalar.activation(yst[:], ps_y[:], AF.Identity,
                                             bias=bo_sb[:, mo, :], scale=1.0)
                        nc.sync.dma_start(
                            out=yT[mo * P:(mo + 1) * P, col:col + NCHUNK],
                            in_=yst[:])

    nc.finalize()
    return nc


def _get_state():
    if _STATE:
        return _STATE
    nc = _build()
    bass2jax.install_neuronx_cc_hook()

    partition_name = nc.partition_id_tensor.name if nc.partition_id_tensor else None
    in_names, out_names, out_avals = [], [], []
    for alloc in nc.m.functions[0].allocations:
        if not isinstance(alloc, mybir.MemoryLocationSet):
            continue
        name = alloc.memorylocations[0].name
        if alloc.kind == "ExternalInput":
            if name != partition_name:
                in_names.append(name)
        elif alloc.kind == "ExternalOutput":
            out_names.append(name)
            out_avals.append(jax.core.ShapedArray(
                tuple(alloc.tensor_shape), mybir.dt.np(alloc.dtype)))
    n_params = len(in_names)
    n_outs = len(out_avals)
    in_names_all = in_names + out_names
    if partition_name is not None:
        in_names_all.append(partition_name)

    devices = jax.devices()[:N]
    mesh = Mesh(np.asarray(devices), ("core",))
    shard0 = NamedSharding(mesh, PartitionSpec("core"))

    def _body(*args):
        operands = list(args)
        if partition_name is not None:
            operands.append(bass2jax.partition_id_tensor())
        outs = bass2jax._bass_exec_p.bind(
            *operands,
            out_avals=tuple(out_avals),
            in_names=tuple(in_names_all),
            out_names=tuple(out_names),
            lowering_input_output_aliases=(),
            sim_require_finite=True,
            sim_require_nnan=True,
            nc=nc,
        )
        return tuple(outs)

    donate = tuple(range(n_params, n_params + n_outs))
    jit_bass = jax.jit(
        shard_map(_body, mesh=mesh,
                  in_specs=(PartitionSpec("core"),) * (n_params + n_outs),
                  out_specs=(PartitionSpec("core"),) * n_outs,
                  check_rep=False),
        donate_argnums=donate, keep_unused=True,
    )

    # x: (BT, D) f16 token-sharded -> transpose + all-gather -> per-core full
    # xT (D, BT), stacked to the (N*D, BT) global the bass program expects.
    # Also emits the zeroed output buffer the bass program's donation needs,
    # so no separate dispatch (and no host upload) for it.
    def _gather(xl):
        return (jax.lax.all_gather(jnp.transpose(xl), "core", axis=1, tiled=True),
                jnp.zeros((D, BT), jnp.float32))

    jit_gather = jax.jit(
        shard_map(_gather, mesh=mesh,
                  in_specs=PartitionSpec("core", None),
                  out_specs=(PartitionSpec("core"), PartitionSpec("core")),
                  check_rep=False))

    # yT partials (N*D, BT) -> on-device sum, each core keeps a D/N row
    # slice, transposed so the pulled global is y_flat (BT, D), fp16.
    def _reduce(yl):
        ys = jax.lax.psum_scatter(yl, "core", scatter_dimension=0, tiled=True)
        return jnp.transpose(ys).astype(jnp.float16)

    jit_reduce = jax.jit(
        shard_map(_reduce, mesh=mesh,
                  in_specs=PartitionSpec("core"),
                  out_specs=PartitionSpec(None, "core"), check_rep=False))

    _STATE.update(
        nc=nc, mesh=mesh, shard0=shard0, in_names=in_names,
        jit_bass=jit_bass, jit_gather=jit_gather, jit_reduce=jit_reduce,
    )
    return _STATE


def _host_prep(Wq, bq, Wk, bk, Wv, bv, Wo, bo):
    """Per-core fp16 weight slices, concatenated core-major for shard_map."""
    Wq, Wk, Wv, Wo = (np.asarray(a, np.float32) for a in (Wq, Wk, Wv, Wo))
    bq, bk, bv, bo = (np.asarray(a, np.float32) for a in (bq, bk, bv, bo))

    wq_cat = np.empty((N * D, QF), np.float16)
    wkv_cat = np.empty((N * D, P), np.float16)
    wo_cat = np.empty((N * QF, D), np.float16)
    bq_cat = np.empty((N * QF, 1), np.float32)
    bqn_cat = np.empty((N * QF, 1), np.float32)
    bkv_cat = np.empty((N * P, 1), np.float32)
    bkvn_cat = np.empty((N * P, 1), np.float32)
    bo_cat = np.zeros((N * D, 1), np.float32)
    for c in range(N):
        qs = slice(c * QF, (c + 1) * QF)
        ks = slice(c * KF, (c + 1) * KF)
        wq_cat[c * D:(c + 1) * D] = Wq[qs, :].T
        wkv_cat[c * D:(c + 1) * D, 0:KF] = Wk[ks, :].T
        wkv_cat[c * D:(c + 1) * D, KF:P] = Wv[ks, :].T
        wo_cat[c * QF:(c + 1) * QF] = Wo[:, qs].T
        bq_c = bq[qs].reshape(QF, 1)
        bq_cat[c * QF:(c + 1) * QF] = bq_c
        bqn_cat[c * QF:(c + 1) * QF] = -bq_c
        bkv_c = np.concatenate([bk[ks], bv[ks]]).reshape(P, 1)
        bkv_cat[c * P:(c + 1) * P] = bkv_c
        bkvn_cat[c * P:(c + 1) * P] = -bkv_c
    bo_cat[0:D] = bo.reshape(D, 1)
    return {
        "wqT": wq_cat, "wkvT": wkv_cat, "woT": wo_cat,
        "bq": bq_cat, "bqn": bqn_cat, "bkv": bkv_cat, "bkvn": bkvn_cat,
        "bo": bo_cat,
    }


def _run(x_flat, weight_args):
    """x upload + gather dispatch first, then weight prep/upload overlaps it."""
    st = _get_state()
    mesh = st["mesh"]
    xsh = NamedSharding(mesh, PartitionSpec("core", None))
    x_dev = jax.device_put(x_flat, xsh)
    xg, zeros = st["jit_gather"](x_dev)
    cats = _host_prep(**weight_args)
    names = [n for n in st["in_names"] if n != "xT"]
    put = jax.device_put([cats[n] for n in names], [st["shard0"]] * len(names))
    by_name = dict(zip(names, put))
    by_name["xT"] = xg
    args = [by_name[n] for n in st["in_names"]] + [zeros]
    (ypart,) = st["jit_bass"](*args)
    return np.asarray(st["jit_reduce"](ypart))     # (BT, D) f16


def kernel(x, Wq, bq, Wk, bk, Wv, bv, Wo, bo):
    x_flat = np.ascontiguousarray(
        np.asarray(x, np.float32).reshape(BT, D).astype(np.float16))
    y_flat = _run(x_flat, dict(Wq=Wq, bq=bq, Wk=Wk, bk=bk, Wv=Wv, bv=bv,
                               Wo=Wo, bo=bo))
    return y_flat.astype(np.float32).reshape(B, T, D)
